# revision 13
# baseline (speedup 1.0000x reference)
"""Trainium2 Bass kernel for a 2-layer spiking (snntorch Leaky) net.

reference semantics (per timestep t, 100 steps):
    m1 = be1*m1 + cur1 - s1_prev          # cur1 = x@w1.T + b1 (hoisted)
    s1 = (m1 > 1)
    cur2 = s1 @ w2.T + b2
    m2 = be2*m2 + cur2 - s2_prev
    s2 = (m2 > 1)
    record (s2, m2)
returns (spk2_rec, mem2_rec) each [100, 8192, 10] float32.

Sharding: pure data-parallel over batch (8192 -> 8 cores x 1024).

Layer-1 membrane is kept PSUM-resident in a rescaled form that removes the
per-step cur1 re-feed and the DVE FMA pass entirely:
    u = m1 - p,  p = cur1/(1-be1)   (steady state; update: u' = be1*u - s1)
    F = be1^-(t-t0) * u             (per 25-step segment, rescaled between)
    spike:  s1 = (u > E) = (E*c < F),  E = TH - p,  c = be1^-(t-t0)
    update: F -= be1^-(tau+1) * s1  (PE matmul with fp32-scaled identity)
Per step: one compare pass (DVE/Pool, split by chunks), one accumulate
matmul per 512-col PSUM bank (PE), plus the tiny layer-2 work.
"""

import sys

import numpy as np

try:  # concourse is normally on the default path; add the repo as fallback
    import concourse  # noqa: F401
except ImportError:
    sys.path.insert(0, "/opt/trn_rl_repo")

B, NI, NH, NO = 8192, 784, 300, 10
NCORES = 8
BC = B // NCORES  # 1024
NSTEPS = 100
TH = 1.0
SEG = 25  # segment length for the F rescaling
NKT = 7  # k tiles over NI+1=785 contraction rows (6x128 + 17)
KT = [128] * 6 + [17]
KO = np.cumsum([0] + KT)

_BUILT = {}


def _build(be1, nsteps=NSTEPS):
    """Build the Bass module for one core (SPMD across 8). be1 is baked in."""
    import concourse.bass as bass
    import concourse.mybir as mybir
    from concourse import bacc
    from concourse.tile import TileContext
    from concourse.masks import make_identity

    f32 = mybir.dt.float32
    f32r = mybir.dt.float32r
    f16 = mybir.dt.float16
    AF = mybir.ActivationFunctionType
    OP = mybir.AluOpType

    # per-step scalars (exact fp64 -> fp32 at bake time)
    cval = [float(be1 ** -(t % SEG)) for t in range(nsteps)]
    wval = [float(-(be1 ** -((t % SEG) + 1))) for t in range(nsteps)]
    rs = float(be1**SEG)

    nc = bacc.Bacc("TRN2", target_bir_lowering=False)

    x_d = nc.dram_tensor("x", [BC, NI], f32, kind="ExternalInput")
    w1_d = nc.dram_tensor("w1tp", [NI + 1, NH], f32r, kind="ExternalInput")
    cv_d = nc.dram_tensor("cvals", [128, nsteps], f32, kind="ExternalInput")
    w2_d = nc.dram_tensor("w2all", [301, NO], f32r, kind="ExternalInput")
    be2_d = nc.dram_tensor("be2t", [128, 8 * NO], f32, kind="ExternalInput")
    spk_d = nc.dram_tensor("spk", [nsteps, 8, 128, NO], f16, kind="ExternalOutput")
    mem_d = nc.dram_tensor("mem", [nsteps, 8, 128, NO], f32, kind="ExternalOutput")

    with TileContext(nc) as tc:
        with (
            tc.tile_pool(name="st", bufs=1) as st,
            tc.tile_pool(name="xb", bufs=3) as xbp,
            tc.tile_pool(name="fp", bufs=1, space="PSUM") as fpp,
            tc.tile_pool(name="pt", bufs=2, space="PSUM") as ptp,
            tc.tile_pool(name="p2", bufs=1, space="PSUM") as p2p,
        )        :
            # ---- persistent psum state ----
            F0 = fpp.tile([128, 1024], f32, tag="F0", name="F0")
            F1 = fpp.tile([128, 1024], f32, tag="F1", name="F1")
            F2 = fpp.tile([128, 512], f32, tag="F2", name="F2")
            ps2 = p2p.tile([128, 512], f32, tag="ps2", name="ps2")

            # ---- constants / state tiles ----
            identr = st.tile([128, 128], f32r)
            make_identity(nc, identr[:])
            id32 = st.tile([128, 128], f32)
            make_identity(nc, id32[:])
            id16 = st.tile([128, 128], f16)
            make_identity(nc, id16[:])
            sI2 = st.tile([128, 128], f32r)
            nc.vector.tensor_single_scalar(sI2[:], identr[:], -TH / 2, OP.mult)
            idc = [st.tile([128, 128], f32r, tag=f"idc{i}", name=f"idc{i}") for i in range(2)]

            thb = st.tile([128, 1], f32)
            nc.vector.memset(thb[:], TH)
            nthb = st.tile([128, 1], f32)
            nc.vector.memset(nthb[:], -TH)
            zrow = st.tile([1, 128], f16)
            nc.vector.memset(zrow[:], 0.0)
            zr16 = st.tile([1, 512], f16)
            nc.vector.memset(zr16[:], 0.0)

            cvals = st.tile([128, nsteps], f32)
            nc.sync.dma_start(out=cvals[:], in_=cv_d[:])
            be2t = st.tile([128, 8 * NO], f32)
            nc.sync.dma_start(out=be2t[:], in_=be2_d[:])

            w1s = []
            for k in range(NKT):
                t = st.tile([128, NH], f32r, tag=f"w1_{k}", name=f"w1_{k}")
                nc.sync.dma_start(out=t[0 : KT[k], :], in_=w1_d[KO[k] : KO[k] + KT[k], :])
                w1s.append(t)
            w2ra = st.tile([128, NO], f32r)
            nc.sync.dma_start(out=w2ra[:], in_=w2_d[0:128, :])
            w2rb = st.tile([128, NO], f32r)
            nc.sync.dma_start(out=w2rb[:], in_=w2_d[128:256, :])
            w2re = st.tile([128, NO], f32r)
            nc.sync.dma_start(out=w2re[0:45, :], in_=w2_d[256:301, :])
            w2ro = st.tile([128, NO], f32r)
            nc.sync.dma_start(out=w2ro[64:109, :], in_=w2_d[256:301, :])

            E0 = st.tile([128, 1024], f32)
            E1 = st.tile([128, 1024], f32)
            E2 = st.tile([128, 512], f32)
            sg0 = [st.tile([128, 1024], f16, tag=f"sg0_{i}", name=f"sg0_{i}") for i in range(2)]
            sg1 = [st.tile([128, 1024], f16, tag=f"sg1_{i}", name=f"sg1_{i}") for i in range(2)]
            sg2 = [st.tile([128, 512], f16, tag=f"sg2_{i}", name=f"sg2_{i}") for i in range(2)]
            m2 = [st.tile([128, 8 * NO], f32, tag=f"m2_{i}", name=f"m2_{i}") for i in range(2)]
            m2t = st.tile([128, 8 * NO], f32)
            sgn = [st.tile([128, 8 * NO], f16, tag=f"sgn_{i}", name=f"sgn_{i}") for i in range(2)]
            nc.vector.memset(m2[0][:], 0.0)
            nc.vector.memset(sgn[0][:], -1.0)

            xt = [st.tile([128, BC], f32r, tag=f"xt_{k}", name=f"xt_{k}") for k in range(NKT)]
            # contraction row 784 (bias row of w1tp) is driven by a ones row:
            # fill rows 0..16 with ones; transposed x overwrites rows 0..15
            nc.vector.memset(xt[6][0:17, :], 1.0)

            # ---- PE warmup burst so the clock-gate opens before cur1 ----
            for wi in range(50):
                nc.tensor.matmul(
                    ps2[0:128, 0:128],
                    lhsT=id16[:],
                    rhs=id16[:],
                    start=(wi == 0),
                    stop=(wi == 49),
                )

            # F2 zero-init (rows 0..88) so later accumulates have a started group
            nc.tensor.matmul(
                F2[0:109, 0:512],
                lhsT=zrow[0:1, 0:109],
                rhs=zr16[0:1, :],
                start=True,
                stop=False,
            )

            # ---- load x, transpose, cur1 -> F psum (scaled by -1/(1-be1)) ----
            cpe = [nc.scalar.copy, nc.vector.tensor_copy, nc.gpsimd.tensor_copy]
            for jb in range(2):
                for i in range(4 * jb, 4 * (jb + 1)):
                    xb = xbp.tile([128, NI], f32)
                    nc.sync.dma_start(out=xb[:], in_=x_d[128 * i : 128 * (i + 1), :])
                    for k in range(NKT):
                        kk = KT[k] if k < 6 else 16  # x has only 784 cols
                        tp = ptp.tile([128, 512], f32, tag="tp", name="tp")
                        nc.tensor.transpose(
                            tp[0:kk, 0:128], xb[:, KO[k] : KO[k] + kk], id32[:]
                        )
                        cpe[(i * NKT + k) % 3](
                            xt[k][0:kk, 128 * i : 128 * (i + 1)], tp[0:kk, 0:128]
                        )
                cs = slice(512 * jb, 512 * (jb + 1))
                for k in range(NKT):
                    nc.tensor.matmul(
                        F0[:, cs],
                        lhsT=w1s[k][0 : KT[k], 0:128],
                        rhs=xt[k][0 : KT[k], cs],
                        start=(k == 0),
                        stop=False,
                    )
                for k in range(NKT):
                    nc.tensor.matmul(
                        F1[:, cs],
                        lhsT=w1s[k][0 : KT[k], 128:256],
                        rhs=xt[k][0 : KT[k], cs],
                        start=(k == 0),
                        stop=False,
                    )
                ftmp = ptp.tile([128, 512], f32, tag="tp", name="ftmp")
                for k in range(NKT):
                    nc.tensor.matmul(
                        ftmp[0:44, 0:512],
                        lhsT=w1s[k][0 : KT[k], 256:300],
                        rhs=xt[k][0 : KT[k], cs],
                        start=(k == 0),
                        stop=(k == NKT - 1),
                    )
                # pack [44, 512] -> F2 parity layout (even j rows 0:44, odd 45:89)
                for q in range(4):
                    j = 4 * jb + q
                    r0 = 1 if j % 2 == 0 else 65
                    eng = nc.vector.tensor_copy if q % 2 else nc.gpsimd.tensor_copy
                    eng(
                        F2[r0 : r0 + 44, 128 * (j // 2) : 128 * (j // 2) + 128],
                        ftmp[0:44, 128 * q : 128 * (q + 1)],
                    )

            # ---- E = TH + F0  (E2 ones-rows poisoned so compare emits 1.0) ----
            nc.scalar.activation(E0[:], F0[:], AF.Identity, bias=thb[:, 0:1], scale=1.0)
            nc.scalar.activation(E1[:], F1[:], AF.Identity, bias=thb[:, 0:1], scale=1.0)
            nc.scalar.activation(E2[0:109, :], F2[0:109, :], AF.Identity, bias=thb[0:109, 0:1], scale=1.0)
            nc.vector.memset(E2[0:1, :], -1e30)
            nc.vector.memset(E2[64:65, :], -1e30)

            # ---- time loop ----
            for t in range(nsteps):
                p, q = t % 2, (t + 1) % 2
                cv = cvals[:, t : t + 1]
                if t < nsteps - 1:
                    nc.scalar.activation(idc[p][:], identr[:], AF.Identity, scale=wval[t])
                # compare: s1 = (E*c < F), split DVE / Pool
                nc.vector.scalar_tensor_tensor(
                    sg0[p][:, 0:512], E0[:, 0:512], cv, F0[:, 0:512], OP.mult, OP.is_lt
                )
                nc.vector.scalar_tensor_tensor(
                    sg0[p][:, 512:1024], E0[:, 512:1024], cv, F0[:, 512:1024], OP.mult, OP.is_lt
                )
                nc.gpsimd.scalar_tensor_tensor(
                    sg1[p][:, 0:512], E1[:, 0:512], cv, F1[:, 0:512], OP.mult, OP.is_lt
                )
                nc.gpsimd.scalar_tensor_tensor(
                    sg1[p][:, 512:1024], E1[:, 512:1024], cv, F1[:, 512:1024], OP.mult, OP.is_lt
                )
                nc.gpsimd.scalar_tensor_tensor(
                    sg2[p][0:109, :],
                    E2[0:109, :],
                    cvals[0:109, t : t + 1],
                    F2[0:109, :],
                    OP.mult,
                    OP.is_lt,
                )
                # layer-2 psum: -TH/2*sgn_prev (start zeroes bank), then cur2+bias
                nc.tensor.matmul(
                    ps2[:, 0:80], lhsT=sI2[:], rhs=sgn[p][:], start=True, stop=False
                )
                # F -= be1^-(tau+1) * s1
                if t < nsteps - 1:
                    st_, sp_ = False, (t == nsteps - 2)
                    nc.tensor.matmul(F0[:, 0:512], lhsT=idc[p][:], rhs=sg0[p][:, 0:512], start=st_, stop=sp_)
                    nc.tensor.matmul(F0[:, 512:1024], lhsT=idc[p][:], rhs=sg0[p][:, 512:1024], start=st_, stop=sp_)
                    nc.tensor.matmul(F1[:, 0:512], lhsT=idc[p][:], rhs=sg1[p][:, 0:512], start=st_, stop=sp_)
                    nc.tensor.matmul(F1[:, 512:1024], lhsT=idc[p][:], rhs=sg1[p][:, 512:1024], start=st_, stop=sp_)
                    nc.tensor.matmul(F2[0:109, :], lhsT=idc[p][0:109, 0:109], rhs=sg2[p][0:109, :], start=st_, stop=sp_)
                # cur2 = s1 @ w2.T + b2 (bias riding the ch2 ones-row)
                for j in range(8):
                    je = 128 * (j // 2)
                    r0 = 0 if j % 2 == 0 else 64
                    w2k2 = w2re if j % 2 == 0 else w2ro
                    nc.tensor.matmul(
                        ps2[:, 10 * j : 10 * j + 10],
                        lhsT=sg2[p][r0 : r0 + 45, je : je + 128],
                        rhs=w2k2[r0 : r0 + 45, :],
                        start=False,
                        stop=False,
                    )
                    nc.tensor.matmul(
                        ps2[:, 10 * j : 10 * j + 10],
                        lhsT=sg0[p][:, 128 * j : 128 * (j + 1)],
                        rhs=w2ra[:],
                        start=False,
                        stop=False,
                    )
                    nc.tensor.matmul(
                        ps2[:, 10 * j : 10 * j + 10],
                        lhsT=sg1[p][:, 128 * j : 128 * (j + 1)],
                        rhs=w2rb[:],
                        start=False,
                        stop=(j == 7),
                    )
                # m2 = be2*m2 + (cur2 + b2 - TH*s2_prev)
                nc.vector.tensor_tensor(m2t[:], m2[p][:], be2t[:], OP.mult)
                nc.vector.tensor_tensor(m2[q][:], m2t[:], ps2[:, 0:80], OP.add)
                nc.scalar.sign(sgn[q][:], m2[q][:], bias=nthb[:, 0:1])
                # outputs
                nc.scalar.dma_start(
                    out=mem_d[t].rearrange("j p h -> p j h"),
                    in_=m2[q][:].rearrange("p (j h) -> p j h", h=NO),
                )
                nc.sync.dma_start(
                    out=spk_d[t].rearrange("j p h -> p j h"),
                    in_=sgn[q][:].rearrange("p (j h) -> p j h", h=NO),
                )
                # segment boundary: F *= be1^SEG so the scale stays bounded
                if t % SEG == SEG - 1 and t < nsteps - 1:
                    nc.vector.tensor_single_scalar(F0[:], F0[:], rs, OP.mult)
                    nc.gpsimd.tensor_single_scalar(F1[:], F1[:], rs, OP.mult)
                    nc.gpsimd.tensor_single_scalar(F2[0:109, :], F2[0:109, :], rs, OP.mult)

    nc.compile()
    return nc


def _prep_inputs(x, w1, b1, beta1, w2, b2, beta2, be1):
    x = np.asarray(x, np.float32)
    w1 = np.asarray(w1, np.float32)
    b1 = np.asarray(b1, np.float32)
    w2 = np.asarray(w2, np.float32)
    b2 = np.asarray(b2, np.float32)
    be2 = np.clip(np.asarray(beta2, np.float32), 0.0, 1.0)

    inv = np.float32(1.0 / max(1.0 - be1, 1e-6))
    w1tp = np.zeros((NI + 1, NH), np.float32)
    w1tp[:NI, :] = -inv * w1.T
    w1tp[NI, :] = -inv * b1

    cvals = np.tile(
        np.array([be1 ** -(t % SEG) for t in range(NSTEPS)], np.float32)[None, :],
        (128, 1),
    )

    w2t = w2.T.astype(np.float32)  # [300, 10]
    b2r = (b2 - np.float32(TH / 2))[None, :]
    w2all = np.concatenate(
        [w2t[0:128], w2t[128:256], w2t[256:300], b2r, b2r, w2t[256:300]], axis=0
    ).astype(np.float32)  # [346, 10]

    be2t = np.tile(be2[None, :], (128, 8)).astype(np.float32)

    shared = {"w1tp": w1tp, "cvals": cvals, "w2all": w2all, "be2t": be2t}
    xs = x.reshape(NCORES, BC, NI)
    return [dict(shared, x=np.ascontiguousarray(xs[i])) for i in range(NCORES)]


def kernel(x, w1, b1, beta1, w2, b2, beta2, _trace=False):
    from concourse import bass_utils

    be1 = float(np.clip(np.asarray(beta1, np.float32)[0], 0.0, 1.0))
    key = ("nc", be1)
    if key not in _BUILT:
        _BUILT[key] = _build(be1)
    nc = _BUILT[key]
    in_maps = _prep_inputs(x, w1, b1, beta1, w2, b2, beta2, be1)
    res = bass_utils.run_bass_kernel_spmd(
        nc, in_maps, core_ids=list(range(NCORES)), trace=_trace
    )
    _BUILT["nc"] = nc  # for test.py's CoreSim timing hook
    _BUILT["last"] = res
    spk = np.concatenate(
        [
            ((r["spk"].astype(np.float32) + 1.0) * 0.5).reshape(NSTEPS, BC, NO)
            for r in res.results
        ],
        axis=1,
    )
    mem = np.concatenate(
        [r["mem"].reshape(NSTEPS, BC, NO) for r in res.results], axis=1
    )
    return spk, mem


# revision 14
# speedup vs baseline: 1.0194x; 1.0194x over previous
"""Trainium2 Bass kernel for a 2-layer spiking (snntorch Leaky) net.

reference semantics (per timestep t, 100 steps):
    m1 = be1*m1 + cur1 - s1_prev          # cur1 = x@w1.T + b1 (hoisted)
    s1 = (m1 > 1)
    cur2 = s1 @ w2.T + b2
    m2 = be2*m2 + cur2 - s2_prev
    s2 = (m2 > 1)
    record (s2, m2)
returns (spk2_rec, mem2_rec) each [100, 8192, 10] float32.

Sharding: pure data-parallel over batch (8192 -> 8 cores x 1024).

Layer-1 membrane is kept PSUM-resident in a rescaled form that removes the
per-step cur1 re-feed and the DVE FMA pass entirely:
    u = m1 - p,  p = cur1/(1-be1)   (steady state; update: u' = be1*u - s1)
    F = be1^-(t-t0) * u             (per 25-step segment, rescaled between)
    spike:  s1 = (u > E) = (E*c < F),  E = TH - p,  c = be1^-(t-t0)
    update: F -= be1^-(tau+1) * s1  (PE matmul with fp32-scaled identity)
Per step: one compare pass (DVE/Pool, split by chunks), one accumulate
matmul per 512-col PSUM bank (PE), plus the tiny layer-2 work.
"""

import sys

import numpy as np

try:  # concourse is normally on the default path; add the repo as fallback
    import concourse  # noqa: F401
except ImportError:
    sys.path.insert(0, "/opt/trn_rl_repo")

B, NI, NH, NO = 8192, 784, 300, 10
NCORES = 8
BC = B // NCORES  # 1024
NSTEPS = 100
TH = 1.0
SEG = 25  # segment length for the F rescaling
NKT = 7  # k tiles over NI+1=785 contraction rows (6x128 + 17)
KT = [128] * 6 + [17]
KO = np.cumsum([0] + KT)

_BUILT = {}


def _build(be1, nsteps=NSTEPS):
    """Build the Bass module for one core (SPMD across 8). be1 is baked in."""
    import concourse.bass as bass
    import concourse.mybir as mybir
    from concourse import bacc
    from concourse.tile import TileContext
    from concourse.masks import make_identity

    f32 = mybir.dt.float32
    f32r = mybir.dt.float32r
    f16 = mybir.dt.float16
    AF = mybir.ActivationFunctionType
    OP = mybir.AluOpType

    # per-step scalars (exact fp64 -> fp32 at bake time)
    cval = [float(be1 ** -(t % SEG)) for t in range(nsteps)]
    wval = [float(-(be1 ** -((t % SEG) + 1))) for t in range(nsteps)]
    rs = float(be1**SEG)

    nc = bacc.Bacc("TRN2", target_bir_lowering=False)

    x_d = nc.dram_tensor("x", [BC, NI], f32, kind="ExternalInput")
    w1_d = nc.dram_tensor("w1tp", [NI + 1, NH], f32r, kind="ExternalInput")
    cv_d = nc.dram_tensor("cvals", [128, nsteps], f32, kind="ExternalInput")
    w2_d = nc.dram_tensor("w2all", [301, NO], f32r, kind="ExternalInput")
    be2_d = nc.dram_tensor("be2t", [128, 8 * NO], f32, kind="ExternalInput")
    spk_d = nc.dram_tensor("spk", [nsteps, 8, 128, NO], f16, kind="ExternalOutput")
    mem_d = nc.dram_tensor("mem", [nsteps, 8, 128, NO], f32, kind="ExternalOutput")

    with TileContext(nc) as tc:
        with (
            tc.tile_pool(name="st", bufs=1) as st,
            tc.tile_pool(name="xb", bufs=3) as xbp,
            tc.tile_pool(name="fp", bufs=1, space="PSUM") as fpp,
            tc.tile_pool(name="pt", bufs=2, space="PSUM") as ptp,
            tc.tile_pool(name="p2", bufs=1, space="PSUM") as p2p,
        )        :
            # ---- persistent psum state ----
            F0 = fpp.tile([128, 1024], f32, tag="F0", name="F0")
            F1 = fpp.tile([128, 1024], f32, tag="F1", name="F1")
            F2 = fpp.tile([128, 512], f32, tag="F2", name="F2")
            ps2 = p2p.tile([128, 512], f32, tag="ps2", name="ps2")

            # ---- constants / state tiles ----
            identr = st.tile([128, 128], f32r)
            make_identity(nc, identr[:])
            id32 = st.tile([128, 128], f32)
            make_identity(nc, id32[:])
            id16 = st.tile([128, 128], f16)
            make_identity(nc, id16[:])
            sI2 = st.tile([128, 128], f32r)
            nc.vector.tensor_single_scalar(sI2[:], identr[:], -TH / 2, OP.mult)
            idc = [st.tile([128, 128], f32r, tag=f"idc{i}", name=f"idc{i}") for i in range(2)]

            thb = st.tile([128, 1], f32)
            nc.vector.memset(thb[:], TH)
            nthb = st.tile([128, 1], f32)
            nc.vector.memset(nthb[:], -TH)
            zrow = st.tile([1, 128], f16)
            nc.vector.memset(zrow[:], 0.0)
            zr16 = st.tile([1, 512], f16)
            nc.vector.memset(zr16[:], 0.0)

            cvals = st.tile([128, nsteps], f32)
            nc.sync.dma_start(out=cvals[:], in_=cv_d[:])
            be2t = st.tile([128, 8 * NO], f32)
            nc.sync.dma_start(out=be2t[:], in_=be2_d[:])

            w1s = []
            for k in range(NKT):
                t = st.tile([128, NH], f32r, tag=f"w1_{k}", name=f"w1_{k}")
                nc.sync.dma_start(out=t[0 : KT[k], :], in_=w1_d[KO[k] : KO[k] + KT[k], :])
                w1s.append(t)
            w2ra = st.tile([128, NO], f32r)
            nc.sync.dma_start(out=w2ra[:], in_=w2_d[0:128, :])
            w2rb = st.tile([128, NO], f32r)
            nc.sync.dma_start(out=w2rb[:], in_=w2_d[128:256, :])
            w2re = st.tile([128, NO], f32r)
            nc.sync.dma_start(out=w2re[0:45, :], in_=w2_d[256:301, :])
            w2ro = st.tile([128, NO], f32r)
            nc.sync.dma_start(out=w2ro[64:109, :], in_=w2_d[256:301, :])

            E0 = st.tile([128, 1024], f32)
            E1 = st.tile([128, 1024], f32)
            E2 = st.tile([128, 512], f32)
            sg0 = [st.tile([128, 1024], f16, tag=f"sg0_{i}", name=f"sg0_{i}") for i in range(2)]
            sg1 = [st.tile([128, 1024], f16, tag=f"sg1_{i}", name=f"sg1_{i}") for i in range(2)]
            sg2 = [st.tile([128, 512], f16, tag=f"sg2_{i}", name=f"sg2_{i}") for i in range(2)]
            m2 = [st.tile([128, 8 * NO], f32, tag=f"m2_{i}", name=f"m2_{i}") for i in range(2)]
            m2t = st.tile([128, 8 * NO], f32)
            sgn = [st.tile([128, 8 * NO], f16, tag=f"sgn_{i}", name=f"sgn_{i}") for i in range(2)]
            nc.vector.memset(m2[0][:], 0.0)
            nc.vector.memset(sgn[0][:], -1.0)

            xt = [st.tile([128, BC], f32r, tag=f"xt_{k}", name=f"xt_{k}") for k in range(NKT)]
            # contraction row 784 (bias row of w1tp) is driven by a ones row:
            # fill rows 0..16 with ones; transposed x overwrites rows 0..15
            nc.vector.memset(xt[6][0:17, :], 1.0)

            # ---- PE warmup burst so the clock-gate opens before cur1 ----
            for wi in range(50):
                nc.tensor.matmul(
                    ps2[0:128, 0:128],
                    lhsT=id16[:],
                    rhs=id16[:],
                    start=(wi == 0),
                    stop=(wi == 49),
                )

            # F2 zero-init (rows 0..88) so later accumulates have a started group
            nc.tensor.matmul(
                F2[0:109, 0:512],
                lhsT=zrow[0:1, 0:109],
                rhs=zr16[0:1, :],
                start=True,
                stop=False,
            )

            # ---- load x, transpose, cur1 -> F psum (scaled by -1/(1-be1)) ----
            cpe = [nc.scalar.copy, nc.vector.tensor_copy, nc.gpsimd.tensor_copy]
            for jb in range(2):
                for i in range(4 * jb, 4 * (jb + 1)):
                    xb = xbp.tile([128, NI], f32)
                    nc.sync.dma_start(out=xb[:], in_=x_d[128 * i : 128 * (i + 1), :])
                    for k in range(NKT):
                        kk = KT[k] if k < 6 else 16  # x has only 784 cols
                        tp = ptp.tile([128, 512], f32, tag="tp", name="tp")
                        nc.tensor.transpose(
                            tp[0:kk, 0:128], xb[:, KO[k] : KO[k] + kk], id32[:]
                        )
                        cpe[(i * NKT + k) % 3](
                            xt[k][0:kk, 128 * i : 128 * (i + 1)], tp[0:kk, 0:128]
                        )
                cs = slice(512 * jb, 512 * (jb + 1))
                for k in range(NKT):
                    nc.tensor.matmul(
                        F0[:, cs],
                        lhsT=w1s[k][0 : KT[k], 0:128],
                        rhs=xt[k][0 : KT[k], cs],
                        start=(k == 0),
                        stop=False,
                    )
                for k in range(NKT):
                    nc.tensor.matmul(
                        F1[:, cs],
                        lhsT=w1s[k][0 : KT[k], 128:256],
                        rhs=xt[k][0 : KT[k], cs],
                        start=(k == 0),
                        stop=False,
                    )
                ftmp = ptp.tile([128, 512], f32, tag="tp", name="ftmp")
                for k in range(NKT):
                    nc.tensor.matmul(
                        ftmp[0:44, 0:512],
                        lhsT=w1s[k][0 : KT[k], 256:300],
                        rhs=xt[k][0 : KT[k], cs],
                        start=(k == 0),
                        stop=(k == NKT - 1),
                    )
                # pack [44, 512] -> F2 parity layout (even j rows 0:44, odd 45:89)
                for q in range(4):
                    j = 4 * jb + q
                    r0 = 1 if j % 2 == 0 else 65
                    eng = nc.vector.tensor_copy if q % 2 else nc.gpsimd.tensor_copy
                    eng(
                        F2[r0 : r0 + 44, 128 * (j // 2) : 128 * (j // 2) + 128],
                        ftmp[0:44, 128 * q : 128 * (q + 1)],
                    )

            # ---- E = TH + F0  (E2 ones-rows poisoned so compare emits 1.0) ----
            nc.scalar.activation(E0[:], F0[:], AF.Identity, bias=thb[:, 0:1], scale=1.0)
            nc.scalar.activation(E1[:], F1[:], AF.Identity, bias=thb[:, 0:1], scale=1.0)
            nc.scalar.activation(E2[0:109, :], F2[0:109, :], AF.Identity, bias=thb[0:109, 0:1], scale=1.0)
            nc.vector.memset(E2[0:1, :], -1e30)
            nc.vector.memset(E2[64:65, :], -1e30)

            # ---- time loop ----
            for t in range(nsteps):
                p, q = t % 2, (t + 1) % 2
                cv = cvals[:, t : t + 1]
                if t < nsteps - 1:
                    nc.scalar.activation(idc[p][:], identr[:], AF.Identity, scale=wval[t])
                # compare: s1 = (E*c < F), split DVE / Pool
                nc.vector.scalar_tensor_tensor(
                    sg0[p][:, 0:512], E0[:, 0:512], cv, F0[:, 0:512], OP.mult, OP.is_lt
                )
                nc.vector.scalar_tensor_tensor(
                    sg0[p][:, 512:1024], E0[:, 512:1024], cv, F0[:, 512:1024], OP.mult, OP.is_lt
                )
                nc.gpsimd.scalar_tensor_tensor(
                    sg1[p][:, 0:512], E1[:, 0:512], cv, F1[:, 0:512], OP.mult, OP.is_lt
                )
                nc.gpsimd.scalar_tensor_tensor(
                    sg1[p][:, 512:1024], E1[:, 512:1024], cv, F1[:, 512:1024], OP.mult, OP.is_lt
                )
                nc.gpsimd.scalar_tensor_tensor(
                    sg2[p][0:109, :],
                    E2[0:109, :],
                    cvals[0:109, t : t + 1],
                    F2[0:109, :],
                    OP.mult,
                    OP.is_lt,
                )
                # F -= be1^-(tau+1) * s1
                if t < nsteps - 1:
                    st_, sp_ = False, (t == nsteps - 2)
                    nc.tensor.matmul(F0[:, 0:512], lhsT=idc[p][:], rhs=sg0[p][:, 0:512], start=st_, stop=sp_)
                    nc.tensor.matmul(F0[:, 512:1024], lhsT=idc[p][:], rhs=sg0[p][:, 512:1024], start=st_, stop=sp_)
                    nc.tensor.matmul(F1[:, 0:512], lhsT=idc[p][:], rhs=sg1[p][:, 0:512], start=st_, stop=sp_)
                    nc.tensor.matmul(F1[:, 512:1024], lhsT=idc[p][:], rhs=sg1[p][:, 512:1024], start=st_, stop=sp_)
                    nc.tensor.matmul(F2[0:109, :], lhsT=idc[p][0:109, 0:109], rhs=sg2[p][0:109, :], start=st_, stop=sp_)
                # cur2 = s1 @ w2.T + b2 (bias riding the ch2 ones-row).
                # The -TH/2*sgn_prev term goes LAST so the group-opening mm
                # only depends on the early-ready Pool compare, keeping the
                # layer-2 chain off the critical path.
                for j in range(8):
                    je = 128 * (j // 2)
                    r0 = 0 if j % 2 == 0 else 64
                    w2k2 = w2re if j % 2 == 0 else w2ro
                    nc.tensor.matmul(
                        ps2[:, 10 * j : 10 * j + 10],
                        lhsT=sg2[p][r0 : r0 + 45, je : je + 128],
                        rhs=w2k2[r0 : r0 + 45, :],
                        start=(j == 0),
                        stop=False,
                    )
                    nc.tensor.matmul(
                        ps2[:, 10 * j : 10 * j + 10],
                        lhsT=sg0[p][:, 128 * j : 128 * (j + 1)],
                        rhs=w2ra[:],
                        start=False,
                        stop=False,
                    )
                    nc.tensor.matmul(
                        ps2[:, 10 * j : 10 * j + 10],
                        lhsT=sg1[p][:, 128 * j : 128 * (j + 1)],
                        rhs=w2rb[:],
                        start=False,
                        stop=False,
                    )
                nc.tensor.matmul(
                    ps2[:, 0:80], lhsT=sI2[:], rhs=sgn[p][:], start=False, stop=True
                )
                # m2 = be2*m2 + (cur2 + b2 - TH*s2_prev)
                nc.vector.tensor_tensor(m2t[:], m2[p][:], be2t[:], OP.mult)
                nc.vector.tensor_tensor(m2[q][:], m2t[:], ps2[:, 0:80], OP.add)
                nc.scalar.sign(sgn[q][:], m2[q][:], bias=nthb[:, 0:1])
                # outputs
                nc.scalar.dma_start(
                    out=mem_d[t].rearrange("j p h -> p j h"),
                    in_=m2[q][:].rearrange("p (j h) -> p j h", h=NO),
                )
                nc.sync.dma_start(
                    out=spk_d[t].rearrange("j p h -> p j h"),
                    in_=sgn[q][:].rearrange("p (j h) -> p j h", h=NO),
                )
                # segment boundary: F *= be1^SEG so the scale stays bounded
                if t % SEG == SEG - 1 and t < nsteps - 1:
                    nc.vector.tensor_single_scalar(F0[:], F0[:], rs, OP.mult)
                    nc.gpsimd.tensor_single_scalar(F1[:], F1[:], rs, OP.mult)
                    nc.gpsimd.tensor_single_scalar(F2[0:109, :], F2[0:109, :], rs, OP.mult)

    nc.compile()
    return nc


def _prep_inputs(x, w1, b1, beta1, w2, b2, beta2, be1):
    x = np.asarray(x, np.float32)
    w1 = np.asarray(w1, np.float32)
    b1 = np.asarray(b1, np.float32)
    w2 = np.asarray(w2, np.float32)
    b2 = np.asarray(b2, np.float32)
    be2 = np.clip(np.asarray(beta2, np.float32), 0.0, 1.0)

    inv = np.float32(1.0 / max(1.0 - be1, 1e-6))
    w1tp = np.zeros((NI + 1, NH), np.float32)
    w1tp[:NI, :] = -inv * w1.T
    w1tp[NI, :] = -inv * b1

    cvals = np.tile(
        np.array([be1 ** -(t % SEG) for t in range(NSTEPS)], np.float32)[None, :],
        (128, 1),
    )

    w2t = w2.T.astype(np.float32)  # [300, 10]
    b2r = (b2 - np.float32(TH / 2))[None, :]
    w2all = np.concatenate(
        [w2t[0:128], w2t[128:256], w2t[256:300], b2r, b2r, w2t[256:300]], axis=0
    ).astype(np.float32)  # [346, 10]

    be2t = np.tile(be2[None, :], (128, 8)).astype(np.float32)

    shared = {"w1tp": w1tp, "cvals": cvals, "w2all": w2all, "be2t": be2t}
    xs = x.reshape(NCORES, BC, NI)
    return [dict(shared, x=np.ascontiguousarray(xs[i])) for i in range(NCORES)]


def kernel(x, w1, b1, beta1, w2, b2, beta2, _trace=False):
    from concourse import bass_utils

    be1 = float(np.clip(np.asarray(beta1, np.float32)[0], 0.0, 1.0))
    key = ("nc", be1)
    if key not in _BUILT:
        _BUILT[key] = _build(be1)
    nc = _BUILT[key]
    in_maps = _prep_inputs(x, w1, b1, beta1, w2, b2, beta2, be1)
    res = bass_utils.run_bass_kernel_spmd(
        nc, in_maps, core_ids=list(range(NCORES)), trace=_trace
    )
    _BUILT["nc"] = nc  # for test.py's CoreSim timing hook
    _BUILT["last"] = res
    spk = np.concatenate(
        [
            ((r["spk"].astype(np.float32) + 1.0) * 0.5).reshape(NSTEPS, BC, NO)
            for r in res.results
        ],
        axis=1,
    )
    mem = np.concatenate(
        [r["mem"].reshape(NSTEPS, BC, NO) for r in res.results], axis=1
    )
    return spk, mem


# revision 17
# speedup vs baseline: 1.0390x; 1.0193x over previous
"""Trainium2 Bass kernel for a 2-layer spiking (snntorch Leaky) net.

reference semantics (per timestep t, 100 steps):
    m1 = be1*m1 + cur1 - s1_prev          # cur1 = x@w1.T + b1 (hoisted)
    s1 = (m1 > 1)
    cur2 = s1 @ w2.T + b2
    m2 = be2*m2 + cur2 - s2_prev
    s2 = (m2 > 1)
    record (s2, m2)
returns (spk2_rec, mem2_rec) each [100, 8192, 10] float32.

Sharding: pure data-parallel over batch (8192 -> 8 cores x 1024).

Layer-1 membrane is kept PSUM-resident in a rescaled form that removes the
per-step cur1 re-feed and the DVE FMA pass entirely:
    u = m1 - p,  p = cur1/(1-be1)   (steady state; update: u' = be1*u - s1)
    F = be1^-(t-t0) * u             (per 25-step segment, rescaled between)
    spike:  s1 = (u > E) = (E*c < F),  E = TH - p,  c = be1^-(t-t0)
    update: F -= be1^-(tau+1) * s1  (PE matmul with fp32-scaled identity)
Per step: one compare pass (DVE/Pool, split by chunks), one accumulate
matmul per 512-col PSUM bank (PE), plus the tiny layer-2 work.
"""

import sys

import numpy as np

try:  # concourse is normally on the default path; add the repo as fallback
    import concourse  # noqa: F401
except ImportError:
    sys.path.insert(0, "/opt/trn_rl_repo")

B, NI, NH, NO = 8192, 784, 300, 10
NCORES = 8
BC = B // NCORES  # 1024
NSTEPS = 100
TH = 1.0
SEG = 25  # segment length for the F rescaling
NKT = 7  # k tiles over NI+1=785 contraction rows (6x128 + 17)
KT = [128] * 6 + [17]
KO = np.cumsum([0] + KT)

_BUILT = {}


def _build(be1, nsteps=NSTEPS):
    """Build the Bass module for one core (SPMD across 8). be1 is baked in."""
    import concourse.bass as bass
    import concourse.mybir as mybir
    from concourse import bacc
    from concourse.tile import TileContext
    from concourse.masks import make_identity

    f32 = mybir.dt.float32
    f32r = mybir.dt.float32r
    f16 = mybir.dt.float16
    AF = mybir.ActivationFunctionType
    OP = mybir.AluOpType

    # per-step scalars (exact fp64 -> fp32 at bake time)
    cval = [float(be1 ** -(t % SEG)) for t in range(nsteps)]
    wval = [float(-(be1 ** -((t % SEG) + 1))) for t in range(nsteps)]
    rs = float(be1**SEG)

    nc = bacc.Bacc("TRN2", target_bir_lowering=False)

    x_d = nc.dram_tensor("x", [BC, NI], f32, kind="ExternalInput")
    w1_d = nc.dram_tensor("w1tp", [NI + 1, NH], f32r, kind="ExternalInput")
    cv_d = nc.dram_tensor("cvals", [128, nsteps], f32, kind="ExternalInput")
    w2_d = nc.dram_tensor("w2all", [301, NO], f32r, kind="ExternalInput")
    be2_d = nc.dram_tensor("be2t", [128, 8 * NO], f32, kind="ExternalInput")
    spk_d = nc.dram_tensor("spk", [nsteps, 8, 128, NO], f16, kind="ExternalOutput")
    mem_d = nc.dram_tensor("mem", [nsteps, 8, 128, NO], f32, kind="ExternalOutput")

    with TileContext(nc) as tc:
        with (
            tc.tile_pool(name="st", bufs=1) as st,
            tc.tile_pool(name="xb", bufs=3) as xbp,
            tc.tile_pool(name="fp", bufs=1, space="PSUM") as fpp,
            tc.tile_pool(name="pt", bufs=2, space="PSUM") as ptp,
            tc.tile_pool(name="p2", bufs=1, space="PSUM") as p2p,
        )        :
            # ---- persistent psum state ----
            F0 = fpp.tile([128, 1024], f32, tag="F0", name="F0")
            F1 = fpp.tile([128, 1024], f32, tag="F1", name="F1")
            F2 = fpp.tile([128, 512], f32, tag="F2", name="F2")
            ps2 = p2p.tile([128, 512], f32, tag="ps2", name="ps2")

            # ---- constants / state tiles ----
            identr = st.tile([128, 128], f32r)
            make_identity(nc, identr[:])
            id32 = st.tile([128, 128], f32)
            make_identity(nc, id32[:])
            id16 = st.tile([128, 128], f16)
            make_identity(nc, id16[:])
            sI2 = st.tile([128, 128], f32r)
            nc.vector.tensor_single_scalar(sI2[:], identr[:], -TH / 2, OP.mult)
            idc = [st.tile([128, 128], f32r, tag=f"idc{i}", name=f"idc{i}") for i in range(2)]

            thb = st.tile([128, 1], f32)
            nc.vector.memset(thb[:], TH)
            nthb = st.tile([128, 1], f32)
            nc.vector.memset(nthb[:], -TH)
            zrow = st.tile([1, 128], f16)
            nc.vector.memset(zrow[:], 0.0)
            zr16 = st.tile([1, 512], f16)
            nc.vector.memset(zr16[:], 0.0)

            cvals = st.tile([128, nsteps], f32)
            nc.gpsimd.dma_start(out=cvals[:], in_=cv_d[:])
            be2t = st.tile([128, 8 * NO], f32)
            nc.gpsimd.dma_start(out=be2t[:], in_=be2_d[:])

            w1s = []
            for k in range(NKT):
                t = st.tile([128, NH], f32r, tag=f"w1_{k}", name=f"w1_{k}")
                nc.scalar.dma_start(out=t[0 : KT[k], :], in_=w1_d[KO[k] : KO[k] + KT[k], :])
                w1s.append(t)
            w2ra = st.tile([128, NO], f32r)
            nc.gpsimd.dma_start(out=w2ra[:], in_=w2_d[0:128, :])
            w2rb = st.tile([128, NO], f32r)
            nc.gpsimd.dma_start(out=w2rb[:], in_=w2_d[128:256, :])
            w2re = st.tile([128, NO], f32r)
            nc.gpsimd.dma_start(out=w2re[0:45, :], in_=w2_d[256:301, :])
            w2ro = st.tile([128, NO], f32r)
            nc.gpsimd.dma_start(out=w2ro[64:109, :], in_=w2_d[256:301, :])

            E0 = st.tile([128, 1024], f32)
            E1 = st.tile([128, 1024], f32)
            E2 = st.tile([128, 512], f32)
            sg0 = [st.tile([128, 1024], f16, tag=f"sg0_{i}", name=f"sg0_{i}") for i in range(2)]
            sg1 = [st.tile([128, 1024], f16, tag=f"sg1_{i}", name=f"sg1_{i}") for i in range(2)]
            sg2 = [st.tile([128, 512], f16, tag=f"sg2_{i}", name=f"sg2_{i}") for i in range(2)]
            m2 = [st.tile([128, 8 * NO], f32, tag=f"m2_{i}", name=f"m2_{i}") for i in range(2)]
            m2t = st.tile([128, 8 * NO], f32)
            sgn = [st.tile([128, 8 * NO], f16, tag=f"sgn_{i}", name=f"sgn_{i}") for i in range(2)]
            nc.vector.memset(m2[0][:], 0.0)
            nc.vector.memset(sgn[0][:], -1.0)

            xt = [st.tile([128, BC], f32r, tag=f"xt_{k}", name=f"xt_{k}") for k in range(NKT)]
            # contraction row 784 (bias row of w1tp) is driven by a ones row:
            # fill rows 0..16 with ones; transposed x overwrites rows 0..15
            nc.vector.memset(xt[6][0:17, :], 1.0)

            # ---- PE warmup burst so the clock-gate opens before cur1 ----
            for wi in range(50):
                nc.tensor.matmul(
                    ps2[0:128, 0:128],
                    lhsT=id16[:],
                    rhs=id16[:],
                    start=(wi == 0),
                    stop=(wi == 49),
                )

            # F2 zero-init (rows 0..88) so later accumulates have a started group
            nc.tensor.matmul(
                F2[0:109, 0:512],
                lhsT=zrow[0:1, 0:109],
                rhs=zr16[0:1, :],
                start=True,
                stop=False,
            )

            # ---- load x, transpose, cur1 -> F psum (scaled by -1/(1-be1)) ----
            cpe = [nc.scalar.copy, nc.vector.tensor_copy, nc.gpsimd.tensor_copy]
            for jb in range(2):
                for i in range(4 * jb, 4 * (jb + 1)):
                    xb = xbp.tile([128, NI], f32)
                    nc.sync.dma_start(out=xb[:], in_=x_d[128 * i : 128 * (i + 1), :])
                    for k in range(NKT):
                        kk = KT[k] if k < 6 else 16  # x has only 784 cols
                        tp = ptp.tile([128, 512], f32, tag="tp", name="tp")
                        nc.tensor.transpose(
                            tp[0:kk, 0:128], xb[:, KO[k] : KO[k] + kk], id32[:]
                        )
                        cpe[(i * NKT + k) % 3](
                            xt[k][0:kk, 128 * i : 128 * (i + 1)], tp[0:kk, 0:128]
                        )
                cs = slice(512 * jb, 512 * (jb + 1))
                for k in range(NKT):
                    nc.tensor.matmul(
                        F0[:, cs],
                        lhsT=w1s[k][0 : KT[k], 0:128],
                        rhs=xt[k][0 : KT[k], cs],
                        start=(k == 0),
                        stop=False,
                    )
                for k in range(NKT):
                    nc.tensor.matmul(
                        F1[:, cs],
                        lhsT=w1s[k][0 : KT[k], 128:256],
                        rhs=xt[k][0 : KT[k], cs],
                        start=(k == 0),
                        stop=False,
                    )
                ftmp = ptp.tile([128, 512], f32, tag="tp", name="ftmp")
                for k in range(NKT):
                    nc.tensor.matmul(
                        ftmp[0:44, 0:512],
                        lhsT=w1s[k][0 : KT[k], 256:300],
                        rhs=xt[k][0 : KT[k], cs],
                        start=(k == 0),
                        stop=(k == NKT - 1),
                    )
                # pack [44, 512] -> F2 parity layout (even j rows 1:45, odd 65:109)
                for q in range(4):
                    j = 4 * jb + q
                    r0 = 1 if j % 2 == 0 else 65
                    eng = nc.vector.tensor_copy if q % 2 else nc.gpsimd.tensor_copy
                    eng(
                        F2[r0 : r0 + 44, 128 * (j // 2) : 128 * (j // 2) + 128],
                        ftmp[0:44, 128 * q : 128 * (q + 1)],
                    )
                # E = TH + F for this 512-block (ones rows poisoned after)
                nc.scalar.activation(E0[:, cs], F0[:, cs], AF.Identity, bias=thb[:, 0:1], scale=1.0)
                nc.scalar.activation(E1[:, cs], F1[:, cs], AF.Identity, bias=thb[:, 0:1], scale=1.0)
                c2 = slice(256 * jb, 256 * (jb + 1))
                nc.scalar.activation(E2[0:109, c2], F2[0:109, c2], AF.Identity, bias=thb[0:109, 0:1], scale=1.0)
                nc.vector.memset(E2[0:1, c2], -1e30)
                nc.vector.memset(E2[64:65, c2], -1e30)


            # ---- time loop ----
            for t in range(nsteps):
                p, q = t % 2, (t + 1) % 2
                cv = cvals[:, t : t + 1]
                if t < nsteps - 1:
                    nc.scalar.activation(idc[p][:], identr[:], AF.Identity, scale=wval[t])
                # compare: s1 = (E*c < F), split DVE / Pool (sg2 first: it
                # feeds the group-opening cur2 mm and the F2 acc)
                nc.vector.tensor_tensor(m2t[:], m2[p][:], be2t[:], OP.mult)
                nc.gpsimd.scalar_tensor_tensor(
                    sg2[p][0:109, :],
                    E2[0:109, :],
                    cvals[0:109, t : t + 1],
                    F2[0:109, :],
                    OP.mult,
                    OP.is_lt,
                )
                nc.vector.scalar_tensor_tensor(
                    sg0[p][:, 0:512], E0[:, 0:512], cv, F0[:, 0:512], OP.mult, OP.is_lt
                )
                nc.vector.scalar_tensor_tensor(
                    sg0[p][:, 512:1024], E0[:, 512:1024], cv, F0[:, 512:1024], OP.mult, OP.is_lt
                )
                nc.gpsimd.scalar_tensor_tensor(
                    sg1[p][:, 0:512], E1[:, 0:512], cv, F1[:, 0:512], OP.mult, OP.is_lt
                )
                nc.gpsimd.scalar_tensor_tensor(
                    sg1[p][:, 512:1024], E1[:, 512:1024], cv, F1[:, 512:1024], OP.mult, OP.is_lt
                )
                # F -= be1^-(tau+1) * s1
                if t < nsteps - 1:
                    st_, sp_ = False, (t == nsteps - 2)
                    nc.tensor.matmul(F2[0:109, :], lhsT=idc[p][0:109, 0:109], rhs=sg2[p][0:109, :], start=st_, stop=sp_)
                    nc.tensor.matmul(F1[:, 0:512], lhsT=idc[p][:], rhs=sg1[p][:, 0:512], start=st_, stop=sp_)
                    nc.tensor.matmul(F1[:, 512:1024], lhsT=idc[p][:], rhs=sg1[p][:, 512:1024], start=st_, stop=sp_)
                    nc.tensor.matmul(F0[:, 0:512], lhsT=idc[p][:], rhs=sg0[p][:, 0:512], start=st_, stop=sp_)
                    nc.tensor.matmul(F0[:, 512:1024], lhsT=idc[p][:], rhs=sg0[p][:, 512:1024], start=st_, stop=sp_)
                # cur2 = s1 @ w2.T + b2 (bias riding the ch2 ones-row).
                # The -TH/2*sgn_prev term goes LAST so the group-opening mm
                # only depends on the early-ready Pool compare, keeping the
                # layer-2 chain off the critical path.
                for j in range(8):
                    je = 128 * (j // 2)
                    r0 = 0 if j % 2 == 0 else 64
                    w2k2 = w2re if j % 2 == 0 else w2ro
                    nc.tensor.matmul(
                        ps2[:, 10 * j : 10 * j + 10],
                        lhsT=sg2[p][r0 : r0 + 45, je : je + 128],
                        rhs=w2k2[r0 : r0 + 45, :],
                        start=(j == 0),
                        stop=False,
                    )
                    nc.tensor.matmul(
                        ps2[:, 10 * j : 10 * j + 10],
                        lhsT=sg1[p][:, 128 * j : 128 * (j + 1)],
                        rhs=w2rb[:],
                        start=False,
                        stop=False,
                    )
                    nc.tensor.matmul(
                        ps2[:, 10 * j : 10 * j + 10],
                        lhsT=sg0[p][:, 128 * j : 128 * (j + 1)],
                        rhs=w2ra[:],
                        start=False,
                        stop=False,
                    )
                nc.tensor.matmul(
                    ps2[:, 0:80], lhsT=sI2[:], rhs=sgn[p][:], start=False, stop=True
                )
                # m2 = be2*m2 + (cur2 + b2 - TH*s2_prev)
                nc.vector.tensor_tensor(m2[q][:], m2t[:], ps2[:, 0:80], OP.add)
                nc.scalar.sign(sgn[q][:], m2[q][:], bias=nthb[:, 0:1])
                # outputs
                nc.scalar.dma_start(
                    out=mem_d[t].rearrange("j p h -> p j h"),
                    in_=m2[q][:].rearrange("p (j h) -> p j h", h=NO),
                )
                nc.sync.dma_start(
                    out=spk_d[t].rearrange("j p h -> p j h"),
                    in_=sgn[q][:].rearrange("p (j h) -> p j h", h=NO),
                )
                # segment boundary: F *= be1^SEG so the scale stays bounded
                if t % SEG == SEG - 1 and t < nsteps - 1:
                    nc.vector.tensor_single_scalar(F0[:], F0[:], rs, OP.mult)
                    nc.gpsimd.tensor_single_scalar(F1[:], F1[:], rs, OP.mult)
                    nc.gpsimd.tensor_single_scalar(F2[0:109, :], F2[0:109, :], rs, OP.mult)

    nc.compile()
    return nc


def _prep_inputs(x, w1, b1, beta1, w2, b2, beta2, be1):
    x = np.asarray(x, np.float32)
    w1 = np.asarray(w1, np.float32)
    b1 = np.asarray(b1, np.float32)
    w2 = np.asarray(w2, np.float32)
    b2 = np.asarray(b2, np.float32)
    be2 = np.clip(np.asarray(beta2, np.float32), 0.0, 1.0)

    inv = np.float32(1.0 / max(1.0 - be1, 1e-6))
    w1tp = np.zeros((NI + 1, NH), np.float32)
    w1tp[:NI, :] = -inv * w1.T
    w1tp[NI, :] = -inv * b1

    cvals = np.tile(
        np.array([be1 ** -(t % SEG) for t in range(NSTEPS)], np.float32)[None, :],
        (128, 1),
    )

    w2t = w2.T.astype(np.float32)  # [300, 10]
    b2r = (b2 - np.float32(TH / 2))[None, :]
    w2all = np.concatenate(
        [w2t[0:128], w2t[128:256], w2t[256:300], b2r, b2r, w2t[256:300]], axis=0
    ).astype(np.float32)  # [346, 10]

    be2t = np.tile(be2[None, :], (128, 8)).astype(np.float32)

    shared = {"w1tp": w1tp, "cvals": cvals, "w2all": w2all, "be2t": be2t}
    xs = x.reshape(NCORES, BC, NI)
    return [dict(shared, x=np.ascontiguousarray(xs[i])) for i in range(NCORES)]


def kernel(x, w1, b1, beta1, w2, b2, beta2, _trace=False):
    from concourse import bass_utils

    be1 = float(np.clip(np.asarray(beta1, np.float32)[0], 0.0, 1.0))
    key = ("nc", be1)
    if key not in _BUILT:
        _BUILT[key] = _build(be1)
    nc = _BUILT[key]
    in_maps = _prep_inputs(x, w1, b1, beta1, w2, b2, beta2, be1)
    res = bass_utils.run_bass_kernel_spmd(
        nc, in_maps, core_ids=list(range(NCORES)), trace=_trace
    )
    _BUILT["nc"] = nc  # for test.py's CoreSim timing hook
    _BUILT["last"] = res
    spk = np.concatenate(
        [
            ((r["spk"].astype(np.float32) + 1.0) * 0.5).reshape(NSTEPS, BC, NO)
            for r in res.results
        ],
        axis=1,
    )
    mem = np.concatenate(
        [r["mem"].reshape(NSTEPS, BC, NO) for r in res.results], axis=1
    )
    return spk, mem


# revision 19
# speedup vs baseline: 1.0695x; 1.0293x over previous
"""Trainium2 Bass kernel for a 2-layer spiking (snntorch Leaky) net.

reference semantics (per timestep t, 100 steps):
    m1 = be1*m1 + cur1 - s1_prev          # cur1 = x@w1.T + b1 (hoisted)
    s1 = (m1 > 1)
    cur2 = s1 @ w2.T + b2
    m2 = be2*m2 + cur2 - s2_prev
    s2 = (m2 > 1)
    record (s2, m2)
returns (spk2_rec, mem2_rec) each [100, 8192, 10] float32.

Sharding: pure data-parallel over batch (8192 -> 8 cores x 1024).

Layer-1 membrane is kept PSUM-resident in a rescaled form that removes the
per-step cur1 re-feed and the DVE FMA pass entirely:
    u = m1 - p,  p = cur1/(1-be1)   (steady state; update: u' = be1*u - s1)
    F = be1^-(t-t0) * u             (per 25-step segment, rescaled between)
    spike:  s1 = (u > E) = (E*c < F),  E = TH - p,  c = be1^-(t-t0)
    update: F -= be1^-(tau+1) * s1  (PE matmul with fp32-scaled identity)
Per step: one compare pass (DVE/Pool, split by chunks), one accumulate
matmul per 512-col PSUM bank (PE), plus the tiny layer-2 work.
"""

import sys

import numpy as np

try:  # concourse is normally on the default path; add the repo as fallback
    import concourse  # noqa: F401
except ImportError:
    sys.path.insert(0, "/opt/trn_rl_repo")

B, NI, NH, NO = 8192, 784, 300, 10
NCORES = 8
BC = B // NCORES  # 1024
NSTEPS = 100
TH = 1.0
SEG = 25  # segment length for the F rescaling
NKT = 7  # k tiles over NI+1=785 contraction rows (6x128 + 17)
KT = [128] * 6 + [17]
KO = np.cumsum([0] + KT)

_BUILT = {}


def _build(be1, nsteps=NSTEPS):
    """Build the Bass module for one core (SPMD across 8). be1 is baked in."""
    import concourse.bass as bass
    import concourse.mybir as mybir
    from concourse import bacc
    from concourse.tile import TileContext
    from concourse.masks import make_identity

    f32 = mybir.dt.float32
    f32r = mybir.dt.float32r
    f16 = mybir.dt.float16
    AF = mybir.ActivationFunctionType
    OP = mybir.AluOpType

    # per-step scalars (exact fp64 -> fp32 at bake time)
    cval = [float(be1 ** -(t % SEG)) for t in range(nsteps)]
    wval = [float(-(be1 ** -((t % SEG) + 1))) for t in range(nsteps)]
    rs = float(be1**SEG)

    nc = bacc.Bacc("TRN2", target_bir_lowering=False)

    x_d = nc.dram_tensor("x", [BC, NI], f32r, kind="ExternalInput")
    w1_d = nc.dram_tensor("w1tp", [NI + 1, NH], f32r, kind="ExternalInput")
    cv_d = nc.dram_tensor("cvals", [128, nsteps], f32, kind="ExternalInput")
    w2_d = nc.dram_tensor("w2all", [301, NO], f32r, kind="ExternalInput")
    be2_d = nc.dram_tensor("be2t", [128, 8 * NO], f32, kind="ExternalInput")
    spk_d = nc.dram_tensor("spk", [nsteps, 8, 128, NO], f16, kind="ExternalOutput")
    mem_d = nc.dram_tensor("mem", [nsteps, 8, 128, NO], f32, kind="ExternalOutput")

    with TileContext(nc) as tc:
        with (
            tc.tile_pool(name="st", bufs=1) as st,
            tc.tile_pool(name="xb", bufs=8) as xbp,
            tc.tile_pool(name="fp", bufs=1, space="PSUM") as fpp,
            tc.tile_pool(name="pt", bufs=2, space="PSUM") as ptp,
            tc.tile_pool(name="p2", bufs=1, space="PSUM") as p2p,
        )        :
            # ---- persistent psum state ----
            F0 = fpp.tile([128, 1024], f32, tag="F0", name="F0")
            F1 = fpp.tile([128, 1024], f32, tag="F1", name="F1")
            F2 = fpp.tile([128, 512], f32, tag="F2", name="F2")
            ps2 = p2p.tile([128, 512], f32, tag="ps2", name="ps2")

            # ---- constants / state tiles ----
            identr = st.tile([128, 128], f32r)
            make_identity(nc, identr[:])
            id16 = st.tile([128, 128], f16)
            make_identity(nc, id16[:])
            sI2 = st.tile([128, 128], f32r)
            nc.vector.tensor_single_scalar(sI2[:], identr[:], -TH / 2, OP.mult)
            idc = [st.tile([128, 128], f32r, tag=f"idc{i}", name=f"idc{i}") for i in range(2)]

            thb = st.tile([128, 1], f32)
            nc.vector.memset(thb[:], TH)
            nthb = st.tile([128, 1], f32)
            nc.vector.memset(nthb[:], -TH)
            zrow = st.tile([1, 128], f16)
            nc.vector.memset(zrow[:], 0.0)
            zr16 = st.tile([1, 512], f16)
            nc.vector.memset(zr16[:], 0.0)

            cvals = st.tile([128, nsteps], f32)
            nc.gpsimd.dma_start(out=cvals[:], in_=cv_d[:])
            be2t = st.tile([128, 8 * NO], f32)
            nc.gpsimd.dma_start(out=be2t[:], in_=be2_d[:])

            w1s = []
            for k in range(NKT):
                t = st.tile([128, NH], f32r, tag=f"w1_{k}", name=f"w1_{k}")
                nc.scalar.dma_start(out=t[0 : KT[k], :], in_=w1_d[KO[k] : KO[k] + KT[k], :])
                w1s.append(t)
            w2ra = st.tile([128, NO], f32r)
            nc.gpsimd.dma_start(out=w2ra[:], in_=w2_d[0:128, :])
            w2rb = st.tile([128, NO], f32r)
            nc.gpsimd.dma_start(out=w2rb[:], in_=w2_d[128:256, :])
            w2re = st.tile([128, NO], f32r)
            nc.gpsimd.dma_start(out=w2re[0:45, :], in_=w2_d[256:301, :])
            w2ro = st.tile([128, NO], f32r)
            nc.gpsimd.dma_start(out=w2ro[64:109, :], in_=w2_d[256:301, :])

            E0 = st.tile([128, 1024], f32)
            E1 = st.tile([128, 1024], f32)
            E2 = st.tile([128, 512], f32)
            sg0 = [st.tile([128, 1024], f16, tag=f"sg0_{i}", name=f"sg0_{i}") for i in range(2)]
            sg1 = [st.tile([128, 1024], f16, tag=f"sg1_{i}", name=f"sg1_{i}") for i in range(2)]
            sg2 = [st.tile([128, 512], f16, tag=f"sg2_{i}", name=f"sg2_{i}") for i in range(2)]
            m2 = [st.tile([128, 8 * NO], f32, tag=f"m2_{i}", name=f"m2_{i}") for i in range(2)]
            m2t = st.tile([128, 8 * NO], f32)
            sgn = [st.tile([128, 8 * NO], f16, tag=f"sgn_{i}", name=f"sgn_{i}") for i in range(2)]
            nc.vector.memset(m2[0][:], 0.0)
            nc.vector.memset(sgn[0][:], -1.0)

            xt = [st.tile([128, BC], f32r, tag=f"xt_{k}", name=f"xt_{k}") for k in range(NKT)]
            # contraction row 784 (bias row of w1tp) is driven by a ones row:
            # fill rows 0..16 with ones; transposed x overwrites rows 0..15
            nc.vector.memset(xt[6][0:17, :], 1.0)

            # ---- PE warmup burst so the clock-gate opens before cur1 ----
            for wi in range(15):
                nc.tensor.matmul(
                    ps2[0:128, 0:128],
                    lhsT=id16[:],
                    rhs=id16[:],
                    start=(wi == 0),
                    stop=(wi == 14),
                )

            # F2 zero-init (rows 0..88) so later accumulates have a started group
            nc.tensor.matmul(
                F2[0:109, 0:512],
                lhsT=zrow[0:1, 0:109],
                rhs=zr16[0:1, :],
                start=True,
                stop=False,
            )

            # ---- load x, transpose, cur1 -> F psum (scaled by -1/(1-be1)) ----
            cpe = [nc.scalar.copy, nc.vector.tensor_copy, nc.gpsimd.tensor_copy]
            for jb in range(2):
                for i in range(4 * jb, 4 * (jb + 1)):
                    xb = xbp.tile([128, NI], f32r)
                    dq = [nc.sync, nc.scalar, nc.gpsimd][i % 3]
                    dq.dma_start(out=xb[:], in_=x_d[128 * i : 128 * (i + 1), :])
                    for k in range(NKT):
                        kk = KT[k] if k < 6 else 16  # x has only 784 cols
                        tp = ptp.tile([128, 512], f32r, tag="tp", name="tp")
                        nc.tensor.transpose(
                            tp[0:kk, 0:128], xb[:, KO[k] : KO[k] + kk], identr[:]
                        )
                        cpe[(i * NKT + k) % 3](
                            xt[k][0:kk, 128 * i : 128 * (i + 1)], tp[0:kk, 0:128]
                        )
                cs = slice(512 * jb, 512 * (jb + 1))
                for k in range(NKT):
                    nc.tensor.matmul(
                        F0[:, cs],
                        lhsT=w1s[k][0 : KT[k], 0:128],
                        rhs=xt[k][0 : KT[k], cs],
                        start=(k == 0),
                        stop=False,
                    )
                for k in range(NKT):
                    nc.tensor.matmul(
                        F1[:, cs],
                        lhsT=w1s[k][0 : KT[k], 128:256],
                        rhs=xt[k][0 : KT[k], cs],
                        start=(k == 0),
                        stop=False,
                    )
                ftmp = ps2
                for k in range(NKT):
                    nc.tensor.matmul(
                        ftmp[0:44, 0:512],
                        lhsT=w1s[k][0 : KT[k], 256:300],
                        rhs=xt[k][0 : KT[k], cs],
                        start=(k == 0),
                        stop=(k == NKT - 1),
                    )
                # pack [44, 512] -> F2 parity layout (even j rows 1:45, odd 65:109)
                for q in range(4):
                    j = 4 * jb + q
                    r0 = 1 if j % 2 == 0 else 65
                    eng = nc.vector.tensor_copy if q % 2 else nc.gpsimd.tensor_copy
                    eng(
                        F2[r0 : r0 + 44, 128 * (j // 2) : 128 * (j // 2) + 128],
                        ftmp[0:44, 128 * q : 128 * (q + 1)],
                    )
                # E = TH + F for this 512-block (ones rows poisoned after)
                nc.scalar.activation(E0[:, cs], F0[:, cs], AF.Identity, bias=thb[:, 0:1], scale=1.0)
                nc.scalar.activation(E1[:, cs], F1[:, cs], AF.Identity, bias=thb[:, 0:1], scale=1.0)
                c2 = slice(256 * jb, 256 * (jb + 1))
                nc.scalar.activation(E2[0:109, c2], F2[0:109, c2], AF.Identity, bias=thb[0:109, 0:1], scale=1.0)
                nc.vector.memset(E2[0:1, c2], -1e30)
                nc.vector.memset(E2[64:65, c2], -1e30)


            # ---- time loop ----
            for t in range(nsteps):
                p, q = t % 2, (t + 1) % 2
                cv = cvals[:, t : t + 1]
                if t < nsteps - 1:
                    nc.scalar.activation(idc[p][:], identr[:], AF.Identity, scale=wval[t])
                # compare: s1 = (E*c < F), split DVE / Pool (sg2 first: it
                # feeds the group-opening cur2 mm and the F2 acc)
                nc.vector.tensor_tensor(m2t[:], m2[p][:], be2t[:], OP.mult)
                nc.gpsimd.scalar_tensor_tensor(
                    sg2[p][0:109, :],
                    E2[0:109, :],
                    cvals[0:109, t : t + 1],
                    F2[0:109, :],
                    OP.mult,
                    OP.is_lt,
                )
                nc.vector.scalar_tensor_tensor(
                    sg0[p][:, 0:512], E0[:, 0:512], cv, F0[:, 0:512], OP.mult, OP.is_lt
                )
                nc.vector.scalar_tensor_tensor(
                    sg0[p][:, 512:1024], E0[:, 512:1024], cv, F0[:, 512:1024], OP.mult, OP.is_lt
                )
                nc.gpsimd.scalar_tensor_tensor(
                    sg1[p][:, 0:512], E1[:, 0:512], cv, F1[:, 0:512], OP.mult, OP.is_lt
                )
                nc.gpsimd.scalar_tensor_tensor(
                    sg1[p][:, 512:1024], E1[:, 512:1024], cv, F1[:, 512:1024], OP.mult, OP.is_lt
                )
                # F -= be1^-(tau+1) * s1
                if t < nsteps - 1:
                    st_, sp_ = False, (t == nsteps - 2)
                    nc.tensor.matmul(F2[0:109, :], lhsT=idc[p][0:109, 0:109], rhs=sg2[p][0:109, :], start=st_, stop=sp_)
                    nc.tensor.matmul(F1[:, 0:512], lhsT=idc[p][:], rhs=sg1[p][:, 0:512], start=st_, stop=sp_)
                    nc.tensor.matmul(F1[:, 512:1024], lhsT=idc[p][:], rhs=sg1[p][:, 512:1024], start=st_, stop=sp_)
                    nc.tensor.matmul(F0[:, 0:512], lhsT=idc[p][:], rhs=sg0[p][:, 0:512], start=st_, stop=sp_)
                    nc.tensor.matmul(F0[:, 512:1024], lhsT=idc[p][:], rhs=sg0[p][:, 512:1024], start=st_, stop=sp_)
                # cur2 = s1 @ w2.T + b2 (bias riding the ch2 ones-row).
                # The -TH/2*sgn_prev term goes LAST so the group-opening mm
                # only depends on the early-ready Pool compare, keeping the
                # layer-2 chain off the critical path.
                for j in range(8):
                    je = 128 * (j // 2)
                    r0 = 0 if j % 2 == 0 else 64
                    w2k2 = w2re if j % 2 == 0 else w2ro
                    nc.tensor.matmul(
                        ps2[:, 10 * j : 10 * j + 10],
                        lhsT=sg2[p][r0 : r0 + 45, je : je + 128],
                        rhs=w2k2[r0 : r0 + 45, :],
                        start=(j == 0),
                        stop=False,
                    )
                    nc.tensor.matmul(
                        ps2[:, 10 * j : 10 * j + 10],
                        lhsT=sg1[p][:, 128 * j : 128 * (j + 1)],
                        rhs=w2rb[:],
                        start=False,
                        stop=False,
                    )
                    nc.tensor.matmul(
                        ps2[:, 10 * j : 10 * j + 10],
                        lhsT=sg0[p][:, 128 * j : 128 * (j + 1)],
                        rhs=w2ra[:],
                        start=False,
                        stop=False,
                    )
                nc.tensor.matmul(
                    ps2[:, 0:80], lhsT=sI2[:], rhs=sgn[p][:], start=False, stop=True
                )
                # m2 = be2*m2 + (cur2 + b2 - TH*s2_prev)
                nc.gpsimd.tensor_tensor(m2[q][:], m2t[:], ps2[:, 0:80], OP.add)
                nc.scalar.sign(sgn[q][:], m2[q][:], bias=nthb[:, 0:1])
                # outputs
                nc.scalar.dma_start(
                    out=mem_d[t].rearrange("j p h -> p j h"),
                    in_=m2[q][:].rearrange("p (j h) -> p j h", h=NO),
                )
                nc.sync.dma_start(
                    out=spk_d[t].rearrange("j p h -> p j h"),
                    in_=sgn[q][:].rearrange("p (j h) -> p j h", h=NO),
                )
                # segment boundary: F *= be1^SEG so the scale stays bounded
                if t % SEG == SEG - 1 and t < nsteps - 1:
                    nc.vector.tensor_single_scalar(F0[:], F0[:], rs, OP.mult)
                    nc.gpsimd.tensor_single_scalar(F1[:], F1[:], rs, OP.mult)
                    nc.gpsimd.tensor_single_scalar(F2[0:109, :], F2[0:109, :], rs, OP.mult)

    nc.compile()
    return nc


def _prep_inputs(x, w1, b1, beta1, w2, b2, beta2, be1):
    x = np.asarray(x, np.float32)
    w1 = np.asarray(w1, np.float32)
    b1 = np.asarray(b1, np.float32)
    w2 = np.asarray(w2, np.float32)
    b2 = np.asarray(b2, np.float32)
    be2 = np.clip(np.asarray(beta2, np.float32), 0.0, 1.0)

    inv = np.float32(1.0 / max(1.0 - be1, 1e-6))
    w1tp = np.zeros((NI + 1, NH), np.float32)
    w1tp[:NI, :] = -inv * w1.T
    w1tp[NI, :] = -inv * b1

    cvals = np.tile(
        np.array([be1 ** -(t % SEG) for t in range(NSTEPS)], np.float32)[None, :],
        (128, 1),
    )

    w2t = w2.T.astype(np.float32)  # [300, 10]
    b2r = (b2 - np.float32(TH / 2))[None, :]
    w2all = np.concatenate(
        [w2t[0:128], w2t[128:256], w2t[256:300], b2r, b2r, w2t[256:300]], axis=0
    ).astype(np.float32)  # [346, 10]

    be2t = np.tile(be2[None, :], (128, 8)).astype(np.float32)

    shared = {"w1tp": w1tp, "cvals": cvals, "w2all": w2all, "be2t": be2t}
    xs = x.reshape(NCORES, BC, NI)
    return [dict(shared, x=np.ascontiguousarray(xs[i])) for i in range(NCORES)]


def kernel(x, w1, b1, beta1, w2, b2, beta2, _trace=False):
    from concourse import bass_utils

    be1 = float(np.clip(np.asarray(beta1, np.float32)[0], 0.0, 1.0))
    key = ("nc", be1)
    if key not in _BUILT:
        _BUILT[key] = _build(be1)
    nc = _BUILT[key]
    in_maps = _prep_inputs(x, w1, b1, beta1, w2, b2, beta2, be1)
    res = bass_utils.run_bass_kernel_spmd(
        nc, in_maps, core_ids=list(range(NCORES)), trace=_trace
    )
    _BUILT["nc"] = nc  # for test.py's CoreSim timing hook
    _BUILT["last"] = res
    spk = np.concatenate(
        [
            ((r["spk"].astype(np.float32) + 1.0) * 0.5).reshape(NSTEPS, BC, NO)
            for r in res.results
        ],
        axis=1,
    )
    mem = np.concatenate(
        [r["mem"].reshape(NSTEPS, BC, NO) for r in res.results], axis=1
    )
    return spk, mem


# revision 20
# speedup vs baseline: 1.0930x; 1.0219x over previous
"""Trainium2 Bass kernel for a 2-layer spiking (snntorch Leaky) net.

reference semantics (per timestep t, 100 steps):
    m1 = be1*m1 + cur1 - s1_prev          # cur1 = x@w1.T + b1 (hoisted)
    s1 = (m1 > 1)
    cur2 = s1 @ w2.T + b2
    m2 = be2*m2 + cur2 - s2_prev
    s2 = (m2 > 1)
    record (s2, m2)
returns (spk2_rec, mem2_rec) each [100, 8192, 10] float32.

Sharding: pure data-parallel over batch (8192 -> 8 cores x 1024).

Layer-1 membrane is kept PSUM-resident in a rescaled form that removes the
per-step cur1 re-feed and the DVE FMA pass entirely:
    u = m1 - p,  p = cur1/(1-be1)   (steady state; update: u' = be1*u - s1)
    F = be1^-(t-t0) * u             (per 25-step segment, rescaled between)
    spike:  s1 = (u > E) = (E*c < F),  E = TH - p,  c = be1^-(t-t0)
    update: F -= be1^-(tau+1) * s1  (PE matmul with fp32-scaled identity)
Per step: one compare pass (DVE/Pool, split by chunks), one accumulate
matmul per 512-col PSUM bank (PE), plus the tiny layer-2 work.
"""

import sys

import numpy as np

try:  # concourse is normally on the default path; add the repo as fallback
    import concourse  # noqa: F401
except ImportError:
    sys.path.insert(0, "/opt/trn_rl_repo")

B, NI, NH, NO = 8192, 784, 300, 10
NCORES = 8
BC = B // NCORES  # 1024
NSTEPS = 100
TH = 1.0
SEG = 25  # segment length for the F rescaling
NKT = 7  # k tiles over NI+1=785 contraction rows (6x128 + 17)
KT = [128] * 6 + [17]
KO = np.cumsum([0] + KT)

_BUILT = {}


def _build(be1, nsteps=NSTEPS):
    """Build the Bass module for one core (SPMD across 8). be1 is baked in."""
    import concourse.bass as bass
    import concourse.mybir as mybir
    from concourse import bacc
    from concourse.tile import TileContext
    from concourse.masks import make_identity

    f32 = mybir.dt.float32
    f32r = mybir.dt.float32r
    f16 = mybir.dt.float16
    AF = mybir.ActivationFunctionType
    OP = mybir.AluOpType

    # per-step scalars (exact fp64 -> fp32 at bake time)
    cval = [float(be1 ** -(t % SEG)) for t in range(nsteps)]
    wval = [float(-(be1 ** -((t % SEG) + 1))) for t in range(nsteps)]
    rs = float(be1**SEG)

    nc = bacc.Bacc("TRN2", target_bir_lowering=False)

    x_d = nc.dram_tensor("x", [BC, NI], f32r, kind="ExternalInput")
    w1_d = nc.dram_tensor("w1tp", [NI + 1, NH], f32r, kind="ExternalInput")
    cv_d = nc.dram_tensor("cvals", [128, nsteps], f32, kind="ExternalInput")
    w2_d = nc.dram_tensor("w2all", [301, NO], f32r, kind="ExternalInput")
    be2_d = nc.dram_tensor("be2t", [128, 8 * NO], f32, kind="ExternalInput")
    spk_d = nc.dram_tensor("spk", [nsteps, 8, 128, NO], f16, kind="ExternalOutput")
    mem_d = nc.dram_tensor("mem", [nsteps, 8, 128, NO], f32, kind="ExternalOutput")

    with TileContext(nc) as tc:
        with (
            tc.tile_pool(name="st", bufs=1) as st,
            tc.tile_pool(name="xb", bufs=8) as xbp,
            tc.tile_pool(name="fp", bufs=1, space="PSUM") as fpp,
            tc.tile_pool(name="pt", bufs=2, space="PSUM") as ptp,
            tc.tile_pool(name="p2", bufs=1, space="PSUM") as p2p,
        )        :
            # ---- persistent psum state ----
            F0 = fpp.tile([128, 1024], f32, tag="F0", name="F0")
            F1 = fpp.tile([128, 1024], f32, tag="F1", name="F1")
            F2 = fpp.tile([128, 512], f32, tag="F2", name="F2")
            ps2 = p2p.tile([128, 512], f32, tag="ps2", name="ps2")

            # ---- constants / state tiles ----
            identr = st.tile([128, 128], f32r)
            make_identity(nc, identr[:])
            id16 = st.tile([128, 128], f16)
            make_identity(nc, id16[:])
            sI2 = st.tile([128, 128], f32r)
            nc.vector.tensor_single_scalar(sI2[:], identr[:], -TH / 2, OP.mult)
            idc = [st.tile([128, 128], f32r, tag=f"idc{i}", name=f"idc{i}") for i in range(2)]

            thb = st.tile([128, 1], f32)
            nc.vector.memset(thb[:], TH)
            nthb = st.tile([128, 1], f32)
            nc.vector.memset(nthb[:], -TH)
            zrow = st.tile([1, 128], f16)
            nc.vector.memset(zrow[:], 0.0)
            zr16 = st.tile([1, 512], f16)
            nc.vector.memset(zr16[:], 0.0)

            cvals = st.tile([128, nsteps], f32)
            nc.gpsimd.dma_start(out=cvals[:], in_=cv_d[:])
            be2t = st.tile([128, 8 * NO], f32)
            nc.gpsimd.dma_start(out=be2t[:], in_=be2_d[:])

            w1s = []
            for k in range(NKT):
                t = st.tile([128, NH], f32r, tag=f"w1_{k}", name=f"w1_{k}")
                nc.sync.dma_start(out=t[0 : KT[k], :], in_=w1_d[KO[k] : KO[k] + KT[k], :])
                w1s.append(t)
            w2ra = st.tile([128, NO], f32r)
            nc.gpsimd.dma_start(out=w2ra[:], in_=w2_d[0:128, :])
            w2rb = st.tile([128, NO], f32r)
            nc.gpsimd.dma_start(out=w2rb[:], in_=w2_d[128:256, :])
            w2re = st.tile([128, NO], f32r)
            nc.gpsimd.dma_start(out=w2re[0:45, :], in_=w2_d[256:301, :])
            w2ro = st.tile([128, NO], f32r)
            nc.gpsimd.dma_start(out=w2ro[64:109, :], in_=w2_d[256:301, :])

            E0 = st.tile([128, 1024], f32)
            E1 = st.tile([128, 1024], f32)
            E2 = st.tile([128, 512], f32)
            sg0 = [st.tile([128, 1024], f16, tag=f"sg0_{i}", name=f"sg0_{i}") for i in range(2)]
            sg1 = [st.tile([128, 1024], f16, tag=f"sg1_{i}", name=f"sg1_{i}") for i in range(2)]
            sg2 = [st.tile([128, 512], f16, tag=f"sg2_{i}", name=f"sg2_{i}") for i in range(2)]
            m2 = [st.tile([128, 8 * NO], f32, tag=f"m2_{i}", name=f"m2_{i}") for i in range(2)]
            m2t = st.tile([128, 8 * NO], f32)
            sgn = [st.tile([128, 8 * NO], f16, tag=f"sgn_{i}", name=f"sgn_{i}") for i in range(2)]
            nc.vector.memset(m2[0][:], 0.0)
            nc.vector.memset(sgn[0][:], -1.0)

            xt = [st.tile([128, BC], f32r, tag=f"xt_{k}", name=f"xt_{k}") for k in range(NKT)]
            # contraction row 784 (bias row of w1tp) is driven by a ones row:
            # fill rows 0..16 with ones; transposed x overwrites rows 0..15
            nc.vector.memset(xt[6][0:17, :], 1.0)

            # ---- PE warmup burst so the clock-gate opens before cur1 ----
            for wi in range(15):
                nc.tensor.matmul(
                    ps2[0:128, 0:128],
                    lhsT=id16[:],
                    rhs=id16[:],
                    start=(wi == 0),
                    stop=(wi == 14),
                )

            # F2 zero-init (rows 0..88) so later accumulates have a started group
            nc.tensor.matmul(
                F2[0:109, 0:512],
                lhsT=zrow[0:1, 0:109],
                rhs=zr16[0:1, :],
                start=True,
                stop=False,
            )

            # ---- load x, transpose, cur1 -> F psum (scaled by -1/(1-be1)) ----
            cpe = [nc.vector.tensor_copy, nc.gpsimd.tensor_copy, nc.scalar.copy]
            for jb in range(2):
                xbs = []
                for i in range(4 * jb, 4 * (jb + 1)):
                    xb = xbp.tile([128, NI], f32r)
                    dq = [nc.scalar, nc.gpsimd, nc.scalar, nc.gpsimd][i % 4]
                    dq.dma_start(out=xb[:], in_=x_d[128 * i : 128 * (i + 1), :])
                    xbs.append(xb)
                for k in range(NKT):
                    kk = KT[k] if k < 6 else 16  # x has only 784 cols
                    tp = ptp.tile([128, 512], f32r, tag="tp", name="tp")
                    for q in range(4):
                        nc.tensor.transpose(
                            tp[0:kk, 128 * q : 128 * (q + 1)],
                            xbs[q][:, KO[k] : KO[k] + kk],
                            identr[:],
                        )
                    cpe[k % 3](
                        xt[k][0:kk, 512 * jb : 512 * (jb + 1)], tp[0:kk, 0:512]
                    )
                cs = slice(512 * jb, 512 * (jb + 1))
                for k in range(NKT):
                    nc.tensor.matmul(
                        F0[:, cs],
                        lhsT=w1s[k][0 : KT[k], 0:128],
                        rhs=xt[k][0 : KT[k], cs],
                        start=(k == 0),
                        stop=False,
                    )
                for k in range(NKT):
                    nc.tensor.matmul(
                        F1[:, cs],
                        lhsT=w1s[k][0 : KT[k], 128:256],
                        rhs=xt[k][0 : KT[k], cs],
                        start=(k == 0),
                        stop=False,
                    )
                ftmp = ps2
                for k in range(NKT):
                    nc.tensor.matmul(
                        ftmp[0:44, 0:512],
                        lhsT=w1s[k][0 : KT[k], 256:300],
                        rhs=xt[k][0 : KT[k], cs],
                        start=(k == 0),
                        stop=(k == NKT - 1),
                    )
                # pack [44, 512] -> F2 parity layout (even j rows 1:45, odd 65:109)
                for q in range(4):
                    j = 4 * jb + q
                    r0 = 1 if j % 2 == 0 else 65
                    eng = nc.vector.tensor_copy if q % 2 else nc.gpsimd.tensor_copy
                    eng(
                        F2[r0 : r0 + 44, 128 * (j // 2) : 128 * (j // 2) + 128],
                        ftmp[0:44, 128 * q : 128 * (q + 1)],
                    )
                # E = TH + F for this 512-block (ones rows poisoned after)
                nc.scalar.activation(E0[:, cs], F0[:, cs], AF.Identity, bias=thb[:, 0:1], scale=1.0)
                nc.scalar.activation(E1[:, cs], F1[:, cs], AF.Identity, bias=thb[:, 0:1], scale=1.0)
                c2 = slice(256 * jb, 256 * (jb + 1))
                nc.scalar.activation(E2[0:109, c2], F2[0:109, c2], AF.Identity, bias=thb[0:109, 0:1], scale=1.0)
                nc.vector.memset(E2[0:1, c2], -1e30)
                nc.vector.memset(E2[64:65, c2], -1e30)


            # ---- time loop ----
            for t in range(nsteps):
                p, q = t % 2, (t + 1) % 2
                cv = cvals[:, t : t + 1]
                if t < nsteps - 1:
                    nc.scalar.activation(idc[p][:], identr[:], AF.Identity, scale=wval[t])
                # compare: s1 = (E*c < F), split DVE / Pool (sg2 first: it
                # feeds the group-opening cur2 mm and the F2 acc)
                nc.vector.tensor_tensor(m2t[:], m2[p][:], be2t[:], OP.mult)
                nc.gpsimd.scalar_tensor_tensor(
                    sg2[p][0:109, :],
                    E2[0:109, :],
                    cvals[0:109, t : t + 1],
                    F2[0:109, :],
                    OP.mult,
                    OP.is_lt,
                )
                nc.vector.scalar_tensor_tensor(
                    sg0[p][:, 0:512], E0[:, 0:512], cv, F0[:, 0:512], OP.mult, OP.is_lt
                )
                nc.vector.scalar_tensor_tensor(
                    sg0[p][:, 512:1024], E0[:, 512:1024], cv, F0[:, 512:1024], OP.mult, OP.is_lt
                )
                nc.gpsimd.scalar_tensor_tensor(
                    sg1[p][:, 0:512], E1[:, 0:512], cv, F1[:, 0:512], OP.mult, OP.is_lt
                )
                nc.gpsimd.scalar_tensor_tensor(
                    sg1[p][:, 512:1024], E1[:, 512:1024], cv, F1[:, 512:1024], OP.mult, OP.is_lt
                )
                # F -= be1^-(tau+1) * s1
                if t < nsteps - 1:
                    st_, sp_ = False, (t == nsteps - 2)
                    nc.tensor.matmul(F2[0:109, :], lhsT=idc[p][0:109, 0:109], rhs=sg2[p][0:109, :], start=st_, stop=sp_)
                    nc.tensor.matmul(F1[:, 0:512], lhsT=idc[p][:], rhs=sg1[p][:, 0:512], start=st_, stop=sp_)
                    nc.tensor.matmul(F1[:, 512:1024], lhsT=idc[p][:], rhs=sg1[p][:, 512:1024], start=st_, stop=sp_)
                    nc.tensor.matmul(F0[:, 0:512], lhsT=idc[p][:], rhs=sg0[p][:, 0:512], start=st_, stop=sp_)
                    nc.tensor.matmul(F0[:, 512:1024], lhsT=idc[p][:], rhs=sg0[p][:, 512:1024], start=st_, stop=sp_)
                # cur2 = s1 @ w2.T + b2 (bias riding the ch2 ones-row).
                # The -TH/2*sgn_prev term goes LAST so the group-opening mm
                # only depends on the early-ready Pool compare, keeping the
                # layer-2 chain off the critical path.
                for j in range(8):
                    je = 128 * (j // 2)
                    r0 = 0 if j % 2 == 0 else 64
                    w2k2 = w2re if j % 2 == 0 else w2ro
                    nc.tensor.matmul(
                        ps2[:, 10 * j : 10 * j + 10],
                        lhsT=sg2[p][r0 : r0 + 45, je : je + 128],
                        rhs=w2k2[r0 : r0 + 45, :],
                        start=(j == 0),
                        stop=False,
                    )
                    nc.tensor.matmul(
                        ps2[:, 10 * j : 10 * j + 10],
                        lhsT=sg1[p][:, 128 * j : 128 * (j + 1)],
                        rhs=w2rb[:],
                        start=False,
                        stop=False,
                    )
                    nc.tensor.matmul(
                        ps2[:, 10 * j : 10 * j + 10],
                        lhsT=sg0[p][:, 128 * j : 128 * (j + 1)],
                        rhs=w2ra[:],
                        start=False,
                        stop=False,
                    )
                nc.tensor.matmul(
                    ps2[:, 0:80], lhsT=sI2[:], rhs=sgn[p][:], start=False, stop=True
                )
                # m2 = be2*m2 + (cur2 + b2 - TH*s2_prev)
                nc.gpsimd.tensor_tensor(m2[q][:], m2t[:], ps2[:, 0:80], OP.add)
                nc.scalar.sign(sgn[q][:], m2[q][:], bias=nthb[:, 0:1])
                # outputs
                nc.scalar.dma_start(
                    out=mem_d[t].rearrange("j p h -> p j h"),
                    in_=m2[q][:].rearrange("p (j h) -> p j h", h=NO),
                )
                nc.sync.dma_start(
                    out=spk_d[t].rearrange("j p h -> p j h"),
                    in_=sgn[q][:].rearrange("p (j h) -> p j h", h=NO),
                )
                # segment boundary: F *= be1^SEG so the scale stays bounded
                if t % SEG == SEG - 1 and t < nsteps - 1:
                    nc.vector.tensor_single_scalar(F0[:], F0[:], rs, OP.mult)
                    nc.gpsimd.tensor_single_scalar(F1[:], F1[:], rs, OP.mult)
                    nc.gpsimd.tensor_single_scalar(F2[0:109, :], F2[0:109, :], rs, OP.mult)

    nc.compile()
    return nc


def _prep_inputs(x, w1, b1, beta1, w2, b2, beta2, be1):
    x = np.asarray(x, np.float32)
    w1 = np.asarray(w1, np.float32)
    b1 = np.asarray(b1, np.float32)
    w2 = np.asarray(w2, np.float32)
    b2 = np.asarray(b2, np.float32)
    be2 = np.clip(np.asarray(beta2, np.float32), 0.0, 1.0)

    inv = np.float32(1.0 / max(1.0 - be1, 1e-6))
    w1tp = np.zeros((NI + 1, NH), np.float32)
    w1tp[:NI, :] = -inv * w1.T
    w1tp[NI, :] = -inv * b1

    cvals = np.tile(
        np.array([be1 ** -(t % SEG) for t in range(NSTEPS)], np.float32)[None, :],
        (128, 1),
    )

    w2t = w2.T.astype(np.float32)  # [300, 10]
    b2r = (b2 - np.float32(TH / 2))[None, :]
    w2all = np.concatenate(
        [w2t[0:128], w2t[128:256], w2t[256:300], b2r, b2r, w2t[256:300]], axis=0
    ).astype(np.float32)  # [346, 10]

    be2t = np.tile(be2[None, :], (128, 8)).astype(np.float32)

    shared = {"w1tp": w1tp, "cvals": cvals, "w2all": w2all, "be2t": be2t}
    xs = x.reshape(NCORES, BC, NI)
    return [dict(shared, x=np.ascontiguousarray(xs[i])) for i in range(NCORES)]


def kernel(x, w1, b1, beta1, w2, b2, beta2, _trace=False):
    from concourse import bass_utils

    be1 = float(np.clip(np.asarray(beta1, np.float32)[0], 0.0, 1.0))
    key = ("nc", be1)
    if key not in _BUILT:
        _BUILT[key] = _build(be1)
    nc = _BUILT[key]
    in_maps = _prep_inputs(x, w1, b1, beta1, w2, b2, beta2, be1)
    res = bass_utils.run_bass_kernel_spmd(
        nc, in_maps, core_ids=list(range(NCORES)), trace=_trace
    )
    _BUILT["nc"] = nc  # for test.py's CoreSim timing hook
    _BUILT["last"] = res
    spk = np.concatenate(
        [
            ((r["spk"].astype(np.float32) + 1.0) * 0.5).reshape(NSTEPS, BC, NO)
            for r in res.results
        ],
        axis=1,
    )
    mem = np.concatenate(
        [r["mem"].reshape(NSTEPS, BC, NO) for r in res.results], axis=1
    )
    return spk, mem
